# revision 2
# baseline (speedup 1.0000x reference)
"""GuardGCN Trainium2 kernel: 8-core edge-parallel gather/dot pipeline.

Device (Bass, 8 NeuronCores, SPMD): the memory-bound per-edge work —
  L1: pairwise feature dots  sum(x[s]*x[d])  for 500K undirected pairs
  L3: row gathers h0[src] for 1M directed conv edges
  L4: pairwise dots on hidden features h[s]*h[d]
  L6: row gathers h2[src]
Edges are sharded 8 ways, bucketed by 25600-node windows so dma_gather's
int16 relative indices stay in range.

Host: index planning, per-edge scalar chains (thresholds/keep/exp), dense
segment reductions, tiny matmuls.
"""
import os
import sys
sys.path.insert(0, "/opt/trn_rl_repo")
import numpy as np

N = 100000
NPAD = 102400
WIN = 25600
NW = 4
NC = 8
P = 128
NFEAT = 128
NHID = 64
CHUNK = 16384
GCALL = 512


def _wrap_idxs(idx):
    """[n] -> [128, n//16] int16 (i at [i%16, i//16], replicated 8x down)."""
    n = idx.shape[0]
    assert n % 16 == 0
    t = np.zeros((16, n // 16), np.int16)
    ar = np.arange(n)
    t[ar % 16, ar // 16] = idx.astype(np.int16)
    return np.tile(t, (8, 1))


def _unwrap(out2d, nslots, call_len):
    """Invert gather layout: value for slot i of call c is at
    [i%128, c*(call_len//128) + i//128]. Returns [nslots,...]."""
    bc = call_len // 128
    res = np.empty((nslots,) + out2d.shape[2:], out2d.dtype)
    for c in range(nslots // call_len):
        sl = out2d[:, c * bc:(c + 1) * bc]
        il = np.arange(call_len)
        res[c * call_len:(c + 1) * call_len] = sl[il % 128, il // 128]
    return res


def _build_sim_nc(G):
    """Dots kernel: 16 (ws,wd) buckets of G pairs; table [NPAD,128]."""
    from concourse import bacc, mybir, tile
    nc = bacc.Bacc("TRN2", target_bir_lowering=False, debug=False,
                   enable_asserts=True, num_devices=NC)
    dt = mybir.dt.float32
    table = nc.dram_tensor("table", [NPAD, NFEAT], dt, kind="ExternalInput")
    idx_s = nc.dram_tensor("idx_s", [P, 16 * G // 16], mybir.dt.int16,
                           kind="ExternalInput")
    idx_d = nc.dram_tensor("idx_d", [P, 16 * G // 16], mybir.dt.int16,
                           kind="ExternalInput")
    B = G // 128
    dots = nc.dram_tensor("dots", [P, 16 * B], dt, kind="ExternalOutput")
    with tile.TileContext(nc) as tc:
        with tc.tile_pool(name="sb", bufs=2) as sb, \
             tc.tile_pool(name="ix", bufs=1) as ix:
            ist = ix.tile([P, 16 * G // 16], mybir.dt.int16)
            idt = ix.tile([P, 16 * G // 16], mybir.dt.int16)
            nc.sync.dma_start(out=ist[:], in_=idx_s[:])
            nc.sync.dma_start(out=idt[:], in_=idx_d[:])
            W = G // 16
            for b in range(16):
                i, j = b // 4, b % 4
                xs = sb.tile([P, B, NFEAT], dt, tag="xs")
                xd = sb.tile([P, B, NFEAT], dt, tag="xd")
                pr = sb.tile([P, B, NFEAT], dt, tag="pr")
                dd = sb.tile([P, B], dt, tag="dd")
                # SWDGE descriptor ring holds ~1024 descs; stay at 512/call
                for k in range(0, G, GCALL):
                    m = min(GCALL, G - k)
                    nc.gpsimd.dma_gather(
                        xs[:, k // 128:(k + m) // 128, :],
                        table[i * WIN:(i + 1) * WIN, :],
                        ist[:, b * W + k // 16:b * W + (k + m) // 16], m, m,
                        NFEAT)
                    nc.gpsimd.dma_gather(
                        xd[:, k // 128:(k + m) // 128, :],
                        table[j * WIN:(j + 1) * WIN, :],
                        idt[:, b * W + k // 16:b * W + (k + m) // 16], m, m,
                        NFEAT)
                nc.vector.tensor_tensor(out=pr[:], in0=xs[:], in1=xd[:],
                                        op=mybir.AluOpType.mult)
                nc.vector.tensor_reduce(out=dd[:], in_=pr[:],
                                        axis=mybir.AxisListType.X,
                                        op=mybir.AluOpType.add)
                nc.sync.dma_start(out=dots[:, b * B:(b + 1) * B], in_=dd[:])
    nc.compile()
    return nc


def _build_rows_nc(tot):
    """Row-gather kernel: 4 window buckets x (tot//4) edges; table [NPAD,64]."""
    from concourse import bacc, mybir, tile
    nc = bacc.Bacc("TRN2", target_bir_lowering=False, debug=False,
                   enable_asserts=True, num_devices=NC)
    dt = mybir.dt.float32
    table = nc.dram_tensor("table", [NPAD, NHID], dt, kind="ExternalInput")
    idx = nc.dram_tensor("idx", [P, tot // 16], mybir.dt.int16,
                         kind="ExternalInput")
    rows = nc.dram_tensor("rows", [P, tot // 128, NHID], dt,
                          kind="ExternalOutput")
    per = tot // 4
    assert per % CHUNK == 0
    with tile.TileContext(nc) as tc:
        with tc.tile_pool(name="sb", bufs=3) as sb, \
             tc.tile_pool(name="ix", bufs=1) as ix:
            it = ix.tile([P, tot // 16], mybir.dt.int16)
            nc.sync.dma_start(out=it[:], in_=idx[:])
            for w in range(4):
                for c in range(per // GCALL):
                    off = w * per + c * GCALL
                    gt = sb.tile([P, GCALL // 128, NHID], dt, tag="gt")
                    nc.gpsimd.dma_gather(
                        gt[:], table[w * WIN:(w + 1) * WIN, :],
                        it[:, off // 16:(off + GCALL) // 16],
                        GCALL, GCALL, NHID)
                    nc.sync.dma_start(
                        out=rows[:, off // 128:(off + GCALL) // 128, :],
                        in_=gt[:])
    nc.compile()
    return nc


def _plan_buckets(keys, nb, pad_to):
    """Group slots by bucket key; returns (slot_of_pos [nb*pad], valid mask)."""
    order = np.argsort(keys, kind="stable")
    counts = np.bincount(keys, minlength=nb)
    assert counts.max() <= pad_to, (counts.max(), pad_to)
    slot = np.zeros(nb * pad_to, np.int64)
    valid = np.zeros(nb * pad_to, bool)
    pos = 0
    for b in range(nb):
        nbk = counts[b]
        slot[b * pad_to:b * pad_to + nbk] = order[pos:pos + nbk]
        valid[b * pad_to:b * pad_to + nbk] = True
        pos += nbk
    return slot, valid


def _segsum_rows(idx, rows, n):
    order = np.argsort(idx, kind="stable")
    si = idx[order]
    sr = rows[order]
    starts = np.nonzero(np.r_[True, si[1:] != si[:-1]])[0]
    sums = np.add.reduceat(sr, starts, axis=0)
    out = np.zeros((n, rows.shape[1]), rows.dtype)
    out[si[starts]] = sums
    return out


class _Runner:
    def __init__(self):
        self.exec_ns = 0
        self.launches = 0
        self.modules = {}

    def run(self, nc, in_maps):
        from concourse.bass_utils import run_bass_kernel_spmd
        res = run_bass_kernel_spmd(nc, in_maps, core_ids=list(range(NC)))
        self.launches += 1
        self.modules[nc] = self.modules.get(nc, 0) + 1
        if res.exec_time_ns:
            self.exec_ns += res.exec_time_ns
        return res.results


def kernel(x, src, dst, rev, W1, b1, W2, b2, Wd, bd, _runner=None):
    x = np.asarray(x, np.float32)
    src = np.asarray(src, np.int64)
    dst = np.asarray(dst, np.int64)
    rev = np.asarray(rev, np.int64)
    W1 = np.asarray(W1, np.float32); b1 = np.asarray(b1, np.float32)
    W2 = np.asarray(W2, np.float32); b2 = np.asarray(b2, np.float32)
    Wd = np.asarray(Wd, np.float32); bd = np.asarray(bd, np.float32)
    E = src.shape[0]
    n = x.shape[0]
    run = _runner if _runner is not None else _Runner()

    # ---------- host planning ----------
    ar = np.arange(E)
    first = ar < rev
    ps, pd_ = src[first], dst[first]
    pair_of_edge = np.zeros(E, np.int64)
    pair_of_edge[first] = np.arange(first.sum())
    pair_of_edge[rev[first]] = np.arange(first.sum())
    NPAIR = ps.shape[0]
    ppc = -(-NPAIR // NC)            # pairs per core
    ppc = -(-ppc // 128) * 128
    # pad pair list to NC*ppc
    padn = NC * ppc - NPAIR
    ps_p = np.r_[ps, np.zeros(padn, np.int64)]
    pd_p = np.r_[pd_, np.zeros(padn, np.int64)]

    # per-core bucket plans (16 buckets by (s-window, d-window))
    G = 0
    core_plans = []
    for c in range(NC):
        s_c = ps_p[c * ppc:(c + 1) * ppc]
        d_c = pd_p[c * ppc:(c + 1) * ppc]
        keys = (s_c // WIN) * 4 + (d_c // WIN)
        G = max(G, np.bincount(keys, minlength=16).max())
        core_plans.append((s_c, d_c, keys))
    G = int(-(-G // 128) * 128)

    sim_maps = []
    pair_slotmaps = []
    for c in range(NC):
        s_c, d_c, keys = core_plans[c]
        slot, valid = _plan_buckets(keys, 16, G)
        s_rel = np.where(valid, s_c[slot] % WIN, 0)
        d_rel = np.where(valid, d_c[slot] % WIN, 0)
        sim_maps.append({"idx_s": _wrap_idxs(s_rel), "idx_d": _wrap_idxs(d_rel)})
        pair_slotmaps.append((slot, valid))

    # conv planning: 1M directed edges + bucket by src window
    epc = -(-E // NC)
    epad = NC * epc - E
    src_p = np.r_[src, np.zeros(epad, np.int64)]
    CB = 0
    conv_keys = []
    for c in range(NC):
        u = src_p[c * epc:(c + 1) * epc]
        k = u // WIN
        CB = max(CB, np.bincount(k, minlength=4).max())
        conv_keys.append((u, k))
    CB = int(-(-CB // CHUNK) * CHUNK)
    TOT = 4 * CB
    conv_maps = []
    conv_slotmaps = []
    for c in range(NC):
        u, k = conv_keys[c]
        slot, valid = _plan_buckets(k, 4, CB)
        u_rel = np.where(valid, u[slot] % WIN, 0)
        conv_maps.append({"idx": _wrap_idxs(u_rel)})
        conv_slotmaps.append((slot, valid))

    # ---------- build programs ----------
    nc_sim = _build_sim_nc(G)
    nc_rows = _build_rows_nc(TOT)

    def run_sim(table_pad):
        maps = [{**m, "table": table_pad} for m in sim_maps]
        res = run.run(nc_sim, maps)
        dots = np.zeros(NPAIR, np.float32)
        for c in range(NC):
            vals = _unwrap(res[c]["dots"].reshape(P, 16 * (G // 128)), 16 * G, G)
            slot, valid = pair_slotmaps[c]
            gl = slot + c * ppc
            ok = valid & (gl < NPAIR)
            dots[gl[ok]] = vals[ok]
        return dots

    def run_rows(table_pad):
        maps = [{**m, "table": table_pad} for m in conv_maps]
        res = run.run(nc_rows, maps)
        rows = np.zeros((E, NHID), np.float32)
        for c in range(NC):
            vals = _unwrap(res[c]["rows"], TOT, CHUNK)
            slot, valid = conv_slotmaps[c]
            gl = slot + c * epc
            ok = valid & (gl < E)
            rows[gl[ok]] = vals[ok]
        return rows

    # ---------- reference math on host, device for gathers/dots ----------
    def att(feat_pad, dots, mask):
        nrm = np.sqrt((feat_pad[:n] ** 2).sum(1))
        safe = np.where(nrm == 0, 1.0, nrm).astype(np.float32)
        simp = dots / (safe[ps] * safe[pd_])
        sim_e = simp[pair_of_edge]            # symmetric expand to E edges
        sim_e = np.where(sim_e < 0.1, 0.0, sim_e) * mask
        rowsum = np.zeros(n, np.float32)
        np.add.at(rowsum, src, np.abs(sim_e))
        a = sim_e / np.where(rowsum == 0, 1.0, rowsum)[src]
        z = a * Wd[0, 0] + a[rev] * Wd[1, 0] + bd[0]
        keep = 1.0 / (1.0 + np.exp(-z)) > 0.5
        a = np.where(keep, a, 0.0).astype(np.float32)
        deg = np.zeros(n, np.float32)
        np.add.at(deg, src, (a != 0).astype(np.float32))
        lam = 1.0 / (deg + 1.0)
        w_e = np.where(a > 0, np.exp(a), 0.0).astype(np.float32)
        w_s = np.exp(lam).astype(np.float32)
        return w_e, w_s

    def conv(rows_gathered, hh, w_e, w_s, b):
        # deg over [edges ; att self loops ; conv self loops]
        degc = np.zeros(n, np.float32)
        np.add.at(degc, dst, w_e)
        degc += w_s + 1.0
        dis = np.where(degc > 0, degc ** -0.5, 0.0).astype(np.float32)
        normc = dis[src] * w_e * dis[dst]
        agg = _segsum_rows(dst, normc[:, None] * rows_gathered, n)
        agg += (dis * dis * (w_s + 1.0))[:, None] * hh
        return agg + b[None, :]

    xpad = np.zeros((NPAD, NFEAT), np.float32)
    xpad[:n] = x
    dots1 = run_sim(xpad)                                    # L1
    if os.environ.get("K_DUMP"):
        np.save("/tmp/dots1.npy", dots1)
        np.save("/tmp/exp_dots1.npy", (x[ps] * x[pd_]).sum(1).astype(np.float32))
    we1, ws1 = att(xpad, dots1, np.ones(E, np.float32))
    h0 = (x @ W1).astype(np.float32)
    h0pad = np.zeros((NPAD, NHID), np.float32)
    h0pad[:n] = h0
    rows1 = run_rows(h0pad)                                  # L3
    if os.environ.get("K_DUMP"):
        np.save("/tmp/rows1.npy", rows1)
        np.save("/tmp/exp_rows1.npy", h0[src])
    h = np.maximum(conv(rows1, h0, we1, ws1, b1), 0.0).astype(np.float32)

    hpadf = np.zeros((NPAD, NFEAT), np.float32)
    hpadf[:n, :NHID] = h
    dots2 = run_sim(hpadf)                                   # L4
    if os.environ.get("K_DUMP"):
        np.save("/tmp/h.npy", h)
        np.save("/tmp/dots2.npy", dots2)
        np.save("/tmp/exp_dots2.npy", (h[ps] * h[pd_]).sum(1).astype(np.float32))
    we2, ws2 = att(hpadf, dots2, (we1 > 0).astype(np.float32))
    h2 = (h @ W2).astype(np.float32)
    h2pad = np.zeros((NPAD, NHID), np.float32)
    h2pad[:n, :h2.shape[1]] = h2
    rows2 = run_rows(h2pad)                                  # L6
    if os.environ.get("K_DUMP"):
        np.save("/tmp/rows2.npy", rows2)
        np.save("/tmp/exp_rows2.npy", h2[src])
    out = conv(rows2[:, :h2.shape[1]],
               h2, we2, ws2, b2)
    mx = out.max(1, keepdims=True)
    lse = np.log(np.exp(out - mx).sum(1, keepdims=True)) + mx
    return (out - lse).astype(np.float32)



# revision 4
# speedup vs baseline: 2.9861x; 2.9861x over previous
"""GuardGCN Trainium2 kernel: 8-core edge-parallel gather pipeline, v2.

Device (Bass, 8 NeuronCores, SPMD) does the memory-bound per-edge work:
  L1: pairwise feature dots  sum(x[s]*x[d])  for 500K undirected pairs
  L3: row gathers h0[src] for 1M directed conv edges
  L4: pairwise dots on hidden features h[s]*h[d]
  L6: row gathers h2[src]
Host does index planning, the per-edge scalar chains (thresholds/keep/exp)
and the dense segment reductions + tiny matmuls.

Key layout ideas vs the naive one-descriptor-per-edge version:
- Edges/pairs are sharded by CONTIGUOUS src ranges (12500 nodes per core),
  so per-core relative src indices fit dma_gather's int16 directly (no
  src-window bucketing) and the src-sorted slot stream has dense index
  runs: two consecutive slots whose table rows are s and s+delta
  (delta in {0,1}) share ONE overlapping-window descriptor
  (elem_step=F, elem_size=2F) - halving src-side descriptor count.
- dst indices are bucketed into four 25600-row windows (int16 range).
- The layer-1 x table is bf16, so a paired descriptor is 512B (full DMA
  efficiency); 64-wide tables use f32 (256B row stride, the minimum
  dma_gather stride).
- Gathered conv rows are written back in bf16 (half write traffic).
"""
import os
import sys
sys.path.insert(0, "/opt/trn_rl_repo")
import numpy as np
import ml_dtypes

BF16 = ml_dtypes.bfloat16

N = 100000
E = 1000000
NPAIR = 500000
NC = 8
SR = N // NC          # src-shard nodes per core
SRT = SR + 2          # src table rows (+2: overlapping-window pad)
WIN = 25600           # dst index window (int16 range)
NW = 4
DTR = NW * WIN + 2    # dst table rows (+2 overlap pad)
NFEAT = 128
NHID = 64
NCLASS = 40
P = 128
GCALL = 1024          # gather descriptors per call (SWDGE ring limit)


def _r128(v):
    return int(-(-v // 128) * 128)


def _wrap_idxs(idx):
    """[n] -> [128, n//16] int16 (i at [i%16, i//16], replicated 8x down)."""
    n = idx.shape[0]
    assert n % 16 == 0
    t = np.zeros((16, n // 16), np.int16)
    ar = np.arange(n)
    t[ar % 16, ar // 16] = idx.astype(np.int16)
    return np.tile(t, (8, 1))


# ---------------------------------------------------------------- planning

class _SimPlan:
    """Slot plan for the pair-dot launches (shared by L1 and L4).

    Per core the s-desc stream is [w0:P0|P1|S, w1:..., ...] where P0/P1 are
    paired descriptors (two slots, second slot at row offset 0/1) and S are
    singles. The d-desc stream has 2 descs per paired desc, placed so that
    the gathered d tile [P, 2*NP/128, F] elementwise aligns with the s tile
    viewed as [P, NP/128, 2, F]: d-desc for (pair i, half h) sits at stream
    position (2*(i//128)+h)*128 + i%128.
    """

    def __init__(self, ps, pd):
        core = ps // SR
        raw = []
        for c in range(NC):
            sel = np.nonzero(core == c)[0]
            s = ps[sel] - c * SR
            d = pd[sel]
            w = d // WIN
            buckets = []
            for wb in range(NW):
                m = np.nonzero(w == wb)[0]
                o = m[np.argsort(s[m], kind="stable")]
                sb = s[o]
                n = len(o)
                nb = n & ~1
                delta = sb[1:nb:2] - sb[0:nb:2]
                k0 = np.nonzero(delta == 0)[0]
                k1 = np.nonzero(delta == 1)[0]
                kr = np.nonzero(delta >= 2)[0]
                sing = np.concatenate([o[2 * kr], o[2 * kr + 1], o[nb:]])
                sing_s = np.concatenate([sb[2 * kr], sb[2 * kr + 1], sb[nb:]])
                buckets.append({
                    "p0": (sb[2 * k0], sel[o[2 * k0]], sel[o[2 * k0 + 1]]),
                    "p1": (sb[2 * k1], sel[o[2 * k1]], sel[o[2 * k1 + 1]]),
                    "s": (sing_s, sel[sing]),
                })
            raw.append(buckets)
        self.NP0 = [_r128(max(len(raw[c][w]["p0"][0]) for c in range(NC)))
                    for w in range(NW)]
        self.NP1 = [_r128(max(len(raw[c][w]["p1"][0]) for c in range(NC)))
                    for w in range(NW)]
        self.NS = [_r128(max(len(raw[c][w]["s"][0]) for c in range(NC)))
                   for w in range(NW)]
        self.tot_s = sum(self.NP0) + sum(self.NP1) + sum(self.NS)
        self.tot_d = 2 * sum(self.NP0) + 2 * sum(self.NP1) + sum(self.NS)
        totc = self.tot_d // 128
        self.idx_s = np.zeros((NC, self.tot_s), np.int64)
        self.idx_d = np.zeros((NC, self.tot_d), np.int64)
        self.pmap = np.full((NC, P, totc), -1, np.int64)
        for c in range(NC):
            s_off = 0
            d_off = 0
            for wb in range(NW):
                b = raw[c][wb]
                for reg, cnt in (("p0", self.NP0[wb]), ("p1", self.NP1[wb])):
                    svals, ida, idb = b[reg]
                    nr = len(svals)
                    i = np.arange(nr)
                    self.idx_s[c, s_off:s_off + nr] = svals
                    j0 = d_off + 2 * (i // 128) * 128 + (i % 128)
                    j1 = j0 + 128
                    self.idx_d[c, j0] = pd[ida] - wb * WIN
                    self.idx_d[c, j1] = pd[idb] - wb * WIN
                    self.pmap[c, j0 % 128, j0 // 128] = ida
                    self.pmap[c, j1 % 128, j1 // 128] = idb
                    s_off += cnt
                    d_off += 2 * cnt
                svals, ids = b["s"]
                nr = len(svals)
                j = d_off + np.arange(nr)
                self.idx_s[c, s_off:s_off + nr] = svals
                self.idx_d[c, j] = pd[ids] - wb * WIN
                self.pmap[c, j % 128, j // 128] = ids
                s_off += self.NS[wb]
                d_off += self.NS[wb]

    def in_maps(self, s_glob, d_glob):
        """s_glob [NC*SR+2, F], d_glob [DTR, F] (already target dtype)."""
        return [{
            "s_tab": np.ascontiguousarray(s_glob[c * SR:c * SR + SRT]),
            "d_tab": d_glob,
            "idx_s": _wrap_idxs(self.idx_s[c]),
            "idx_d": _wrap_idxs(self.idx_d[c]),
        } for c in range(NC)]

    def unwrap(self, res):
        dots = np.zeros(NPAIR, np.float32)
        for c in range(NC):
            out = np.asarray(res[c]["dots"])
            mm = self.pmap[c]
            v = mm >= 0
            dots[mm[v]] = out[v]
        return dots


class _RowsPlan:
    """Desc plan for the conv row-gather launches (shared by L3 and L6).

    Stream = [paired descs | single descs]; every desc gathers 2 adjacent
    table rows (512B); edge A reads half 0, edge B reads half delta.
    """

    def __init__(self, src):
        core = src // SR
        self.pa = []   # per core: (s_desc_vals, idA, idB, hB, sing_s, idS)
        for c in range(NC):
            sel = np.nonzero(core == c)[0]
            srel = src[sel] - c * SR
            o = np.argsort(srel, kind="stable")
            sb = srel[o]
            n = len(o)
            nb = n & ~1
            delta = sb[1:nb:2] - sb[0:nb:2]
            pk = np.nonzero(delta <= 1)[0]
            rk = np.nonzero(delta >= 2)[0]
            sing = np.concatenate([o[2 * rk], o[2 * rk + 1], o[nb:]])
            sing_s = np.concatenate([sb[2 * rk], sb[2 * rk + 1], sb[nb:]])
            self.pa.append((sb[2 * pk], sel[o[2 * pk]], sel[o[2 * pk + 1]],
                            delta[pk], sing_s, sel[sing]))
        self.NPr = _r128(max(len(p[0]) for p in self.pa))
        self.NSr = _r128(max(len(p[4]) for p in self.pa))
        self.tot = self.NPr + self.NSr
        self.idx = np.zeros((NC, self.tot), np.int64)
        for c in range(NC):
            sv, _, _, _, ss, _ = self.pa[c]
            self.idx[c, :len(sv)] = sv
            self.idx[c, self.NPr:self.NPr + len(ss)] = ss

    def in_maps(self, tab_glob):
        return [{
            "tab": np.ascontiguousarray(tab_glob[c * SR:c * SR + SRT]),
            "idx": _wrap_idxs(self.idx[c]),
        } for c in range(NC)]

    def unwrap(self, res):
        er = np.empty((E, NHID), np.float32)
        for c in range(NC):
            cells = np.asarray(res[c]["rows"]).astype(np.float32)
            flat = cells.transpose(1, 0, 2).reshape(-1, 2, NHID)
            sv, ida, idb, hb, ss, ids = self.pa[c]
            npr = len(sv)
            er[ida] = flat[np.arange(npr), 0]
            er[idb] = flat[np.arange(npr), hb]
            er[ids] = flat[self.NPr + np.arange(len(ss)), 0]
        return er


# ---------------------------------------------------------------- programs

def _build_sim_nc(F, use_bf16, NP0, NP1, NS):
    """Pair-dots program: s gathers (paired + single overlapping-window
    descs) + aligned d gathers + multiply/reduce -> dots [P, tot_d/128]."""
    from concourse import bacc, mybir, tile
    from concourse.ap import AP
    nc = bacc.Bacc("TRN2", target_bir_lowering=False, debug=False,
                   enable_asserts=True, num_devices=NC)
    dt = mybir.dt.bfloat16 if use_bf16 else mybir.dt.float32
    f32 = mybir.dt.float32
    i16 = mybir.dt.int16
    tot_s = sum(NP0) + sum(NP1) + sum(NS)
    tot_d = 2 * sum(NP0) + 2 * sum(NP1) + sum(NS)
    s_tab = nc.dram_tensor("s_tab", [SRT, F], dt, kind="ExternalInput")
    d_tab = nc.dram_tensor("d_tab", [DTR, F], dt, kind="ExternalInput")
    idx_s = nc.dram_tensor("idx_s", [P, tot_s // 16], i16, kind="ExternalInput")
    idx_d = nc.dram_tensor("idx_d", [P, tot_d // 16], i16, kind="ExternalInput")
    dots = nc.dram_tensor("dots", [P, tot_d // 128], f32, kind="ExternalOutput")
    DE = F                        # d elem (one table row)
    s_in = AP(s_tab, 0, [[F, SRT - 1], [1, 2 * F]])
    mult = mybir.AluOpType.mult
    add = mybir.AluOpType.add
    ax = mybir.AxisListType.X
    with tile.TileContext(nc) as tc:
        with tc.tile_pool(name="sb", bufs=3) as sb, \
             tc.tile_pool(name="ix", bufs=1) as ix:
            ist = ix.tile([P, tot_s // 16], i16)
            idt = ix.tile([P, tot_d // 16], i16)
            nc.sync.dma_start(out=ist[:], in_=idx_s[:])
            nc.sync.dma_start(out=idt[:], in_=idx_d[:])
            s_off = 0
            d_off = 0
            for w in range(NW):
                d_in = AP(d_tab, w * WIN * F, [[F, WIN + 1], [1, F]])
                for reg, cnt in (("p0", NP0[w]), ("p1", NP1[w]), ("s", NS[w])):
                    paired = reg != "s"
                    for k in range(0, cnt, GCALL):
                        m = min(GCALL, cnt - k)
                        cols = (2 * m if paired else m) // 128
                        sgt = sb.tile([P, GCALL // 128, 2 * F], dt, tag="sg")
                        dgt = sb.tile([P, 2 * GCALL // 128, DE], dt, tag="dg")
                        prod = sb.tile([P, 2 * GCALL // 128, F], f32, tag="pr")
                        dd = sb.tile([P, 2 * GCALL // 128], f32, tag="dd")
                        nc.gpsimd.dma_gather(
                            sgt[:, :m // 128, :], s_in,
                            ist[:, (s_off + k) // 16:(s_off + k + m) // 16],
                            m, m, 2 * F, elem_step=F)
                        db = d_off + (2 * k if paired else k)
                        nd = 2 * m if paired else m
                        for q in range(0, nd, GCALL):
                            mq = min(GCALL, nd - q)
                            nc.gpsimd.dma_gather(
                                dgt[:, q // 128:(q + mq) // 128, :], d_in,
                                idt[:, (db + q) // 16:(db + q + mq) // 16],
                                mq, mq, DE)
                        if reg == "p0":
                            in0 = sgt[:, :m // 128, 0:F].unsqueeze(2) \
                                .broadcast_to([P, m // 128, 2, F])
                            in1 = dgt[:, :cols, :] \
                                .rearrange("p (c t) f -> p c t f", t=2)
                            po = prod[:, :cols, :] \
                                .rearrange("p (c t) f -> p c t f", t=2)
                        elif reg == "p1":
                            in0 = sgt[:, :m // 128, :] \
                                .rearrange("p c (t f) -> p (c t) f", t=2)
                            in1 = dgt[:, :cols, :]
                            po = prod[:, :cols, :]
                        else:
                            in0 = sgt[:, :m // 128, 0:F]
                            in1 = dgt[:, :cols, :]
                            po = prod[:, :cols, :]
                        nc.any.tensor_tensor(out=po, in0=in0, in1=in1, op=mult)
                        nc.vector.tensor_reduce(
                            out=dd[:, :cols], in_=prod[:, :cols, :],
                            axis=ax, op=add)
                        nc.sync.dma_start(
                            out=dots[:, db // 128:db // 128 + cols],
                            in_=dd[:, :cols])
                    s_off += cnt
                    d_off += 2 * cnt if paired else cnt
    nc.compile()
    return nc


def _build_rows_nc(TOTR):
    """Conv row-gather program: paired/single overlapping-window gathers of
    h rows (f32, 512B descs) -> bf16 cast -> dense write-back."""
    from concourse import bacc, mybir, tile
    from concourse.ap import AP
    nc = bacc.Bacc("TRN2", target_bir_lowering=False, debug=False,
                   enable_asserts=True, num_devices=NC)
    f32 = mybir.dt.float32
    bf16 = mybir.dt.bfloat16
    i16 = mybir.dt.int16
    tab = nc.dram_tensor("tab", [SRT, NHID], f32, kind="ExternalInput")
    idx = nc.dram_tensor("idx", [P, TOTR // 16], i16, kind="ExternalInput")
    rows = nc.dram_tensor("rows", [P, TOTR // 128, 2 * NHID], bf16,
                          kind="ExternalOutput")
    t_in = AP(tab, 0, [[NHID, SRT - 1], [1, 2 * NHID]])
    with tile.TileContext(nc) as tc:
        with tc.tile_pool(name="sb", bufs=3) as sb, \
             tc.tile_pool(name="ix", bufs=1) as ix:
            it = ix.tile([P, TOTR // 16], i16)
            nc.sync.dma_start(out=it[:], in_=idx[:])
            for k in range(0, TOTR, GCALL):
                m = min(GCALL, TOTR - k)
                gt = sb.tile([P, GCALL // 128, 2 * NHID], f32, tag="gt")
                cv = sb.tile([P, GCALL // 128, 2 * NHID], bf16, tag="cv")
                nc.gpsimd.dma_gather(
                    gt[:, :m // 128, :], t_in,
                    it[:, k // 16:(k + m) // 16], m, m, 2 * NHID,
                    elem_step=NHID)
                nc.any.tensor_copy(out=cv[:, :m // 128, :],
                                   in_=gt[:, :m // 128, :])
                nc.sync.dma_start(out=rows[:, k // 128:(k + m) // 128, :],
                                  in_=cv[:, :m // 128, :])
    nc.compile()
    return nc


# ---------------------------------------------------------------- runner

class _Runner:
    def __init__(self):
        self.exec_ns = 0
        self.launches = 0
        self.modules = {}

    def run(self, nc, in_maps):
        from concourse.bass_utils import run_bass_kernel_spmd
        res = run_bass_kernel_spmd(nc, in_maps, core_ids=list(range(NC)))
        self.launches += 1
        self.modules[nc] = self.modules.get(nc, 0) + 1
        if res.exec_time_ns:
            self.exec_ns += res.exec_time_ns
        return res.results


# ---------------------------------------------------------------- kernel

def kernel(x, src, dst, rev, W1, b1, W2, b2, Wd, bd, _runner=None):
    x = np.asarray(x, np.float32)
    src = np.asarray(src, np.int64)
    dst = np.asarray(dst, np.int64)
    rev = np.asarray(rev, np.int64)
    W1 = np.asarray(W1, np.float32); b1 = np.asarray(b1, np.float32)
    W2 = np.asarray(W2, np.float32); b2 = np.asarray(b2, np.float32)
    Wd = np.asarray(Wd, np.float32); bd = np.asarray(bd, np.float32)
    n = x.shape[0]
    run = _runner if _runner is not None else _Runner()

    # ---------- host planning (topology-only; shared across layers) ----------
    ar = np.arange(E)
    first = ar < rev
    ps, pd_ = src[first], dst[first]
    pair_of_edge = np.zeros(E, np.int64)
    pair_of_edge[first] = np.arange(NPAIR)
    pair_of_edge[rev[first]] = np.arange(NPAIR)

    simp = _SimPlan(ps, pd_)
    rowp = _RowsPlan(src)

    nc_sim1 = _build_sim_nc(NFEAT, True, simp.NP0, simp.NP1, simp.NS)
    nc_sim4 = _build_sim_nc(NHID, False, simp.NP0, simp.NP1, simp.NS)
    nc_rows = _build_rows_nc(rowp.tot)

    dst_order = np.argsort(dst, kind="stable")
    ds_sorted = dst[dst_order]
    seg_starts = np.nonzero(np.r_[True, ds_sorted[1:] != ds_sorted[:-1]])[0]
    seg_nodes = ds_sorted[seg_starts]

    def segsum(rows_e):
        sums = np.add.reduceat(rows_e[dst_order], seg_starts, axis=0)
        out = np.zeros((n, rows_e.shape[1]), np.float32)
        out[seg_nodes] = sums
        return out

    def bc(idx, w):
        return np.bincount(idx, weights=w, minlength=n).astype(np.float32)

    # ---------- reference math on host, device for gathers/dots ----------
    def att(feat, dots, mask):
        nrm = np.sqrt((feat ** 2).sum(1))
        safe = np.where(nrm == 0, 1.0, nrm).astype(np.float32)
        sim_p = dots / (safe[ps] * safe[pd_])
        sim_e = sim_p[pair_of_edge]            # symmetric expand to E edges
        sim_e = np.where(sim_e < 0.1, 0.0, sim_e) * mask
        rowsum = bc(src, np.abs(sim_e))
        a = sim_e / np.where(rowsum == 0, 1.0, rowsum)[src]
        z = a * Wd[0, 0] + a[rev] * Wd[1, 0] + bd[0]
        keep = 1.0 / (1.0 + np.exp(-z)) > 0.5
        a = np.where(keep, a, 0.0).astype(np.float32)
        deg = bc(src, (a != 0).astype(np.float32))
        lam = 1.0 / (deg + 1.0)
        w_e = np.where(a > 0, np.exp(a), 0.0).astype(np.float32)
        w_s = np.exp(lam).astype(np.float32)
        return w_e, w_s

    def conv(rows_e, hh, w_e, w_s, b):
        # deg over [edges ; att self loops ; conv self loops]
        degc = bc(dst, w_e) + w_s + 1.0
        dis = np.where(degc > 0, degc ** -0.5, 0.0).astype(np.float32)
        normc = dis[src] * w_e * dis[dst]
        agg = segsum(normc[:, None] * rows_e)
        agg += (dis * dis * (w_s + 1.0))[:, None] * hh
        return agg + b[None, :]

    def pad_glob(t, width):
        g = np.zeros((NC * SR + 2, width), np.float32)
        g[:n, :t.shape[1]] = t
        return g

    def pad_dtab(t, width):
        g = np.zeros((DTR, width), np.float32)
        g[:n, :t.shape[1]] = t
        return g

    # L1: dots on raw features (bf16 tables)
    res = run.run(nc_sim1, simp.in_maps(pad_glob(x, NFEAT).astype(BF16),
                                        pad_dtab(x, NFEAT).astype(BF16)))
    dots1 = simp.unwrap(res)
    if os.environ.get("K_DUMP"):
        np.save("/tmp/dots1.npy", dots1)
        np.save("/tmp/exp_dots1.npy", (x[ps] * x[pd_]).sum(1).astype(np.float32))
    we1, ws1 = att(x, dots1, np.ones(E, np.float32))
    h0 = (x @ W1).astype(np.float32)
    res = run.run(nc_rows, rowp.in_maps(pad_glob(h0, NHID)))        # L3
    rows1 = rowp.unwrap(res)
    if os.environ.get("K_DUMP"):
        np.save("/tmp/rows1.npy", rows1)
        np.save("/tmp/exp_rows1.npy", h0[src])
    h = np.maximum(conv(rows1, h0, we1, ws1, b1), 0.0).astype(np.float32)

    # L4: dots on hidden features (f32 tables)
    res = run.run(nc_sim4, simp.in_maps(pad_glob(h, NHID),
                                        pad_dtab(h, NHID)))
    dots2 = simp.unwrap(res)
    if os.environ.get("K_DUMP"):
        np.save("/tmp/dots2.npy", dots2)
        np.save("/tmp/exp_dots2.npy", (h[ps] * h[pd_]).sum(1).astype(np.float32))
    we2, ws2 = att(h, dots2, (we1 > 0).astype(np.float32))
    h2 = (h @ W2).astype(np.float32)
    res = run.run(nc_rows, rowp.in_maps(pad_glob(h2, NHID)))        # L6
    rows2 = rowp.unwrap(res)[:, :NCLASS]
    out = conv(rows2, h2, we2, ws2, b2)
    mx = out.max(1, keepdims=True)
    lse = np.log(np.exp(out - mx).sum(1, keepdims=True)) + mx
    return (out - lse).astype(np.float32)


# revision 10
# speedup vs baseline: 3.3630x; 1.1262x over previous
"""GuardGCN Trainium2 kernel: 8-core edge-parallel gather pipeline, v2.

Device (Bass, 8 NeuronCores, SPMD) does the memory-bound per-edge work:
  L1: pairwise feature dots  sum(x[s]*x[d])  for 500K undirected pairs
  L3: row gathers h0[src] for 1M directed conv edges
  L4: pairwise dots on hidden features h[s]*h[d]
  L6: row gathers h2[src]
Host does index planning, the per-edge scalar chains (thresholds/keep/exp)
and the dense segment reductions + tiny matmuls.

Key layout ideas vs the naive one-descriptor-per-edge version:
- Edges/pairs are sharded by CONTIGUOUS src ranges (12500 nodes per core),
  so per-core relative src indices fit dma_gather's int16 directly (no
  src-window bucketing) and the src-sorted slot stream has dense index
  runs: two consecutive slots whose table rows are s and s+delta
  (delta in {0,1}) share ONE overlapping-window descriptor
  (elem_step=F, elem_size=2F) - halving src-side descriptor count.
- dst indices are bucketed into four 25600-row windows (int16 range).
- The layer-1 x table is bf16, so a paired descriptor is 512B (full DMA
  efficiency); 64-wide tables use f32 (256B row stride, the minimum
  dma_gather stride).
- Gathered conv rows are written back in bf16 (half write traffic).
"""
import os
import sys
sys.path.insert(0, "/opt/trn_rl_repo")
import numpy as np
import ml_dtypes

BF16 = ml_dtypes.bfloat16

N = 100000
E = 1000000
NPAIR = 500000
NC = 8
SR = N // NC          # src-shard nodes per core
SRT = SR + 2          # src table rows (+2: overlapping-window pad)
WIN = 25600           # dst index window (int16 range)
NW = 4
DTR = NW * WIN + 2    # dst table rows (+2 overlap pad)
NFEAT = 128
NHID = 64
NCLASS = 40
P = 128
GCALL = 1024          # gather descriptors per call (SWDGE ring limit)


def _r128(v):
    return int(-(-v // 128) * 128)


def _wrap_idxs(idx):
    """[n] -> [128, n//16] int16 (i at [i%16, i//16], replicated 8x down)."""
    n = idx.shape[0]
    assert n % 16 == 0
    t = np.zeros((16, n // 16), np.int16)
    ar = np.arange(n)
    t[ar % 16, ar // 16] = idx.astype(np.int16)
    return np.tile(t, (8, 1))


# ---------------------------------------------------------------- planning

class _SimPlan:
    """Slot plan for the pair-dot launches (shared by L1 and L4).

    Per core the s-desc stream is [w0:P0|P1|S, w1:..., ...] where P0/P1 are
    paired descriptors (two slots, second slot at row offset 0/1) and S are
    singles. The d-desc stream has 2 descs per paired desc, placed so that
    the gathered d tile [P, 2*NP/128, F] elementwise aligns with the s tile
    viewed as [P, NP/128, 2, F]: d-desc for (pair i, half h) sits at stream
    position (2*(i//128)+h)*128 + i%128.
    """

    def __init__(self, ps, pd):
        core = ps // SR
        raw = []
        for c in range(NC):
            sel = np.nonzero(core == c)[0]
            s = ps[sel] - c * SR
            d = pd[sel]
            w = d // WIN
            buckets = []
            for wb in range(NW):
                m = np.nonzero(w == wb)[0]
                o = m[np.argsort(s[m], kind="stable")]
                sb = s[o]
                n = len(o)
                nb = n & ~1
                delta = sb[1:nb:2] - sb[0:nb:2]
                k0 = np.nonzero(delta == 0)[0]
                k1 = np.nonzero(delta == 1)[0]
                kr = np.nonzero(delta >= 2)[0]
                sing = np.concatenate([o[2 * kr], o[2 * kr + 1], o[nb:]])
                sing_s = np.concatenate([sb[2 * kr], sb[2 * kr + 1], sb[nb:]])
                buckets.append({
                    "p0": (sb[2 * k0], sel[o[2 * k0]], sel[o[2 * k0 + 1]]),
                    "p1": (sb[2 * k1], sel[o[2 * k1]], sel[o[2 * k1 + 1]]),
                    "s": (sing_s, sel[sing]),
                })
            raw.append(buckets)
        self.NP0 = [_r128(max(len(raw[c][w]["p0"][0]) for c in range(NC)))
                    for w in range(NW)]
        self.NP1 = [_r128(max(len(raw[c][w]["p1"][0]) for c in range(NC)))
                    for w in range(NW)]
        self.NS = [_r128(max(len(raw[c][w]["s"][0]) for c in range(NC)))
                   for w in range(NW)]
        self.tot_s = sum(self.NP0) + sum(self.NP1) + sum(self.NS)
        self.tot_d = 2 * sum(self.NP0) + 2 * sum(self.NP1) + sum(self.NS)
        totc = self.tot_d // 128
        self.idx_s = np.zeros((NC, self.tot_s), np.int64)
        self.idx_d = np.zeros((NC, self.tot_d), np.int64)
        self.pmap = np.full((NC, P, totc), -1, np.int64)
        for c in range(NC):
            s_off = 0
            d_off = 0
            for wb in range(NW):
                b = raw[c][wb]
                for reg, cnt in (("p0", self.NP0[wb]), ("p1", self.NP1[wb])):
                    svals, ida, idb = b[reg]
                    nr = len(svals)
                    i = np.arange(nr)
                    self.idx_s[c, s_off:s_off + nr] = svals
                    j0 = d_off + 2 * (i // 128) * 128 + (i % 128)
                    j1 = j0 + 128
                    self.idx_d[c, j0] = pd[ida] - wb * WIN
                    self.idx_d[c, j1] = pd[idb] - wb * WIN
                    self.pmap[c, j0 % 128, j0 // 128] = ida
                    self.pmap[c, j1 % 128, j1 // 128] = idb
                    s_off += cnt
                    d_off += 2 * cnt
                svals, ids = b["s"]
                nr = len(svals)
                j = d_off + np.arange(nr)
                self.idx_s[c, s_off:s_off + nr] = svals
                self.idx_d[c, j] = pd[ids] - wb * WIN
                self.pmap[c, j % 128, j // 128] = ids
                s_off += self.NS[wb]
                d_off += self.NS[wb]

    def in_maps(self, s_glob, d_glob):
        """s_glob [NC*SR+2, F], d_glob [DTR, F] (already target dtype)."""
        return [{
            "s_tab": np.ascontiguousarray(s_glob[c * SR:c * SR + SRT]),
            "d_tab": d_glob,
            "idx_s": _wrap_idxs(self.idx_s[c]),
            "idx_d": _wrap_idxs(self.idx_d[c]),
        } for c in range(NC)]

    def unwrap(self, res):
        dots = np.zeros(NPAIR, np.float32)
        for c in range(NC):
            out = np.asarray(res[c]["dots"])
            mm = self.pmap[c]
            v = mm >= 0
            dots[mm[v]] = out[v]
        return dots


class _RowsPlan:
    """Desc plan for the conv row-gather launches (shared by L3 and L6).

    Three regions: R0 = same-src pairs (ONE 256B row-desc serves 2 edges),
    R1 = adjacent-src pairs (one 512B 2-row desc), S = singles (256B).
    90% of pairs are same-src, so most descs move and write only one row.
    """

    def __init__(self, src):
        core = src // SR
        self.pa = []   # per core: (s0, idA0, idB0, s1, idA1, idB1, ss, idS)
        for c in range(NC):
            sel = np.nonzero(core == c)[0]
            srel = src[sel] - c * SR
            o = np.argsort(srel, kind="stable")
            sb = srel[o]
            n = len(o)
            nb = n & ~1
            delta = sb[1:nb:2] - sb[0:nb:2]
            k0 = np.nonzero(delta == 0)[0]
            k1 = np.nonzero(delta == 1)[0]
            rk = np.nonzero(delta >= 2)[0]
            sing = np.concatenate([o[2 * rk], o[2 * rk + 1], o[nb:]])
            sing_s = np.concatenate([sb[2 * rk], sb[2 * rk + 1], sb[nb:]])
            self.pa.append((sb[2 * k0], sel[o[2 * k0]], sel[o[2 * k0 + 1]],
                            sb[2 * k1], sel[o[2 * k1]], sel[o[2 * k1 + 1]],
                            sing_s, sel[sing]))
        self.NR0 = _r128(max(len(p[0]) for p in self.pa))
        self.NR1 = _r128(max(len(p[3]) for p in self.pa))
        self.NSs = _r128(max(len(p[6]) for p in self.pa))
        self.idx = np.zeros((NC, self.NR0 + self.NR1 + self.NSs), np.int64)
        for c in range(NC):
            s0, _, _, s1, _, _, ss, _ = self.pa[c]
            self.idx[c, :len(s0)] = s0
            self.idx[c, self.NR0:self.NR0 + len(s1)] = s1
            o = self.NR0 + self.NR1
            self.idx[c, o:o + len(ss)] = ss

    def in_maps(self, tab_glob):
        return [{
            "tab": np.ascontiguousarray(tab_glob[c * SR:c * SR + SRT]),
            "idx": _wrap_idxs(self.idx[c]),
        } for c in range(NC)]

    def unwrap(self, res):
        er = np.empty((E, NHID), np.float32)
        for c in range(NC):
            f0 = np.asarray(res[c]["rows0"]).astype(np.float32) \
                .transpose(1, 0, 2).reshape(-1, NHID)
            f1 = np.asarray(res[c]["rows1"]).astype(np.float32) \
                .transpose(1, 0, 2).reshape(-1, 2, NHID)
            fs = np.asarray(res[c]["rowss"]).astype(np.float32) \
                .transpose(1, 0, 2).reshape(-1, NHID)
            s0, ida0, idb0, s1, ida1, idb1, ss, ids = self.pa[c]
            er[ida0] = f0[:len(s0)]
            er[idb0] = f0[:len(s0)]
            er[ida1] = f1[:len(s1), 0]
            er[idb1] = f1[:len(s1), 1]
            er[ids] = fs[:len(ss)]
        return er


# ---------------------------------------------------------------- programs

def _emit_sim(nc, tc, sb, ix, mybir, AP, F, dt, NP0, NP1, NS,
              s_tab, d_tab, idx_s, idx_d, dots):
    P_ = P
    f32 = mybir.dt.float32
    i16 = mybir.dt.int16
    tot_s = sum(NP0) + sum(NP1) + sum(NS)
    tot_d = 2 * sum(NP0) + 2 * sum(NP1) + sum(NS)
    s_in = AP(s_tab, 0, [[F, SRT - 1], [1, 2 * F]])
    mult = mybir.AluOpType.mult
    add = mybir.AluOpType.add
    ax = mybir.AxisListType.X
    ist = ix.tile([P_, tot_s // 16], i16, tag="ist")
    idt = ix.tile([P_, tot_d // 16], i16, tag="idt")
    nc.sync.dma_start(out=ist[:], in_=idx_s[:])
    nc.sync.dma_start(out=idt[:], in_=idx_d[:])
    s_off = 0
    d_off = 0
    for w in range(NW):
        d_in = AP(d_tab, w * WIN * F, [[F, WIN + 1], [1, F]])
        for reg, cnt in (("p0", NP0[w]), ("p1", NP1[w]), ("s", NS[w])):
            paired = reg != "s"
            for k in range(0, cnt, GCALL):
                m = min(GCALL, cnt - k)
                cols = (2 * m if paired else m) // 128
                sgt = sb.tile([P_, GCALL // 128, 2 * F], dt, tag="sg")
                dgt = sb.tile([P_, 2 * GCALL // 128, F], dt, tag="dg")
                prod = sb.tile([P_, 2 * GCALL // 128, F], f32, tag="pr")
                dd = sb.tile([P_, 2 * GCALL // 128], f32, tag="dd")
                nc.gpsimd.dma_gather(
                    sgt[:, :m // 128, :], s_in,
                    ist[:, (s_off + k) // 16:(s_off + k + m) // 16],
                    m, m, 2 * F, elem_step=F)
                db = d_off + (2 * k if paired else k)
                nd = 2 * m if paired else m
                for q in range(0, nd, GCALL):
                    mq = min(GCALL, nd - q)
                    nc.gpsimd.dma_gather(
                        dgt[:, q // 128:(q + mq) // 128, :], d_in,
                        idt[:, (db + q) // 16:(db + q + mq) // 16],
                        mq, mq, F)
                if reg == "p0":
                    in0 = sgt[:, :m // 128, 0:F].unsqueeze(2) \
                        .broadcast_to([P_, m // 128, 2, F])
                    in1 = dgt[:, :cols, :] \
                        .rearrange("p (c t) f -> p c t f", t=2)
                    po = prod[:, :cols, :] \
                        .rearrange("p (c t) f -> p c t f", t=2)
                elif reg == "p1":
                    in0 = sgt[:, :m // 128, :] \
                        .rearrange("p c (t f) -> p (c t) f", t=2)
                    in1 = dgt[:, :cols, :]
                    po = prod[:, :cols, :]
                else:
                    in0 = sgt[:, :m // 128, 0:F]
                    in1 = dgt[:, :cols, :]
                    po = prod[:, :cols, :]
                nc.any.tensor_tensor(out=po, in0=in0, in1=in1, op=mult)
                nc.vector.tensor_reduce(
                    out=dd[:, :cols], in_=prod[:, :cols, :],
                    axis=ax, op=add)
                nc.sync.dma_start(
                    out=dots[:, db // 128:db // 128 + cols],
                    in_=dd[:, :cols])
            s_off += cnt
            d_off += 2 * cnt if paired else cnt


def _emit_rows(nc, tc, sb, ix, mybir, AP, NR0, NR1, NSs,
               tab, idx, rows0, rows1, rowss):
    P_ = P
    f32 = mybir.dt.float32
    bf16 = mybir.dt.bfloat16
    i16 = mybir.dt.int16
    TOTR = NR0 + NR1 + NSs
    t_in1 = AP(tab, 0, [[NHID, SRT - 1], [1, 2 * NHID]])
    it = ix.tile([P_, TOTR // 16], i16, tag="irt")
    nc.sync.dma_start(out=it[:], in_=idx[:])
    for reg, cnt, off, wid, out in (
            ("r0", NR0, 0, NHID, rows0),
            ("r1", NR1, NR0, 2 * NHID, rows1),
            ("s", NSs, NR0 + NR1, NHID, rowss)):
        for k in range(0, cnt, GCALL):
            m = min(GCALL, cnt - k)
            gt = sb.tile([P_, GCALL // 128, wid], f32, tag=f"gt{wid}")
            cv = sb.tile([P_, GCALL // 128, wid], bf16, tag=f"cv{wid}")
            if reg == "r1":
                nc.gpsimd.dma_gather(
                    gt[:, :m // 128, :], t_in1,
                    it[:, (off + k) // 16:(off + k + m) // 16],
                    m, m, 2 * NHID, elem_step=NHID)
            else:
                nc.gpsimd.dma_gather(
                    gt[:, :m // 128, :], tab[:, :],
                    it[:, (off + k) // 16:(off + k + m) // 16],
                    m, m, NHID)
            nc.any.tensor_copy(out=cv[:, :m // 128, :],
                               in_=gt[:, :m // 128, :])
            nc.sync.dma_start(
                out=out[:, k // 128:(k + m) // 128, :],
                in_=cv[:, :m // 128, :])


def _build_layer_nc(F, use_bf16, sp, rp):
    """One layer = pair-dots (sim) + conv row-gather in a single launch.

    The two pipelines are independent on-device (host combines results), so
    merging them shares the launch ramp/drain and keeps DMA saturated.
    """
    from concourse import bacc, mybir, tile
    from concourse.ap import AP
    nc = bacc.Bacc("TRN2", target_bir_lowering=False, debug=False,
                   enable_asserts=True, num_devices=NC)
    dt = mybir.dt.bfloat16 if use_bf16 else mybir.dt.float32
    f32 = mybir.dt.float32
    bf16 = mybir.dt.bfloat16
    i16 = mybir.dt.int16
    tot_s = sum(sp.NP0) + sum(sp.NP1) + sum(sp.NS)
    tot_d = 2 * sum(sp.NP0) + 2 * sum(sp.NP1) + sum(sp.NS)
    TOTR = rp.NR0 + rp.NR1 + rp.NSs
    s_tab = nc.dram_tensor("s_tab", [SRT, F], dt, kind="ExternalInput")
    d_tab = nc.dram_tensor("d_tab", [DTR, F], dt, kind="ExternalInput")
    idx_s = nc.dram_tensor("idx_s", [P, tot_s // 16], i16,
                           kind="ExternalInput")
    idx_d = nc.dram_tensor("idx_d", [P, tot_d // 16], i16,
                           kind="ExternalInput")
    rtab = nc.dram_tensor("rtab", [SRT, NHID], f32, kind="ExternalInput")
    ridx = nc.dram_tensor("ridx", [P, TOTR // 16], i16,
                          kind="ExternalInput")
    dots = nc.dram_tensor("dots", [P, tot_d // 128], f32,
                          kind="ExternalOutput")
    rows0 = nc.dram_tensor("rows0", [P, rp.NR0 // 128, NHID], bf16,
                           kind="ExternalOutput")
    rows1 = nc.dram_tensor("rows1", [P, max(rp.NR1, 128) // 128, 2 * NHID],
                           bf16, kind="ExternalOutput")
    rowss = nc.dram_tensor("rowss", [P, max(rp.NSs, 128) // 128, NHID],
                           bf16, kind="ExternalOutput")
    with tile.TileContext(nc) as tc:
        with tc.tile_pool(name="sb", bufs=3) as sb, \
             tc.tile_pool(name="ix", bufs=1) as ix:
            _emit_sim(nc, tc, sb, ix, mybir, AP, F, dt,
                      sp.NP0, sp.NP1, sp.NS, s_tab, d_tab, idx_s, idx_d,
                      dots)
            _emit_rows(nc, tc, sb, ix, mybir, AP, rp.NR0, rp.NR1, rp.NSs,
                       rtab, ridx, rows0, rows1, rowss)
    nc.compile()
    return nc


# ---------------------------------------------------------------- runner

class _Runner:
    def __init__(self):
        self.exec_ns = 0
        self.launches = 0
        self.modules = {}

    def run(self, nc, in_maps):
        from concourse.bass_utils import run_bass_kernel_spmd
        res = run_bass_kernel_spmd(nc, in_maps, core_ids=list(range(NC)))
        self.launches += 1
        self.modules[nc] = self.modules.get(nc, 0) + 1
        if res.exec_time_ns:
            self.exec_ns += res.exec_time_ns
        return res.results


# ---------------------------------------------------------------- kernel

def kernel(x, src, dst, rev, W1, b1, W2, b2, Wd, bd, _runner=None):
    x = np.asarray(x, np.float32)
    src = np.asarray(src, np.int64)
    dst = np.asarray(dst, np.int64)
    rev = np.asarray(rev, np.int64)
    W1 = np.asarray(W1, np.float32); b1 = np.asarray(b1, np.float32)
    W2 = np.asarray(W2, np.float32); b2 = np.asarray(b2, np.float32)
    Wd = np.asarray(Wd, np.float32); bd = np.asarray(bd, np.float32)
    n = x.shape[0]
    run = _runner if _runner is not None else _Runner()

    # ---------- host planning (topology-only; shared across layers) ----------
    ar = np.arange(E)
    first = ar < rev
    ps, pd_ = src[first], dst[first]
    pair_of_edge = np.zeros(E, np.int64)
    pair_of_edge[first] = np.arange(NPAIR)
    pair_of_edge[rev[first]] = np.arange(NPAIR)

    simp = _SimPlan(ps, pd_)
    rowp = _RowsPlan(src)

    nc_l1 = _build_layer_nc(NFEAT, True, simp, rowp)
    nc_l2 = _build_layer_nc(NHID, False, simp, rowp)

    dst_order = np.argsort(dst, kind="stable")
    ds_sorted = dst[dst_order]
    seg_starts = np.nonzero(np.r_[True, ds_sorted[1:] != ds_sorted[:-1]])[0]
    seg_nodes = ds_sorted[seg_starts]

    def segsum(rows_e):
        sums = np.add.reduceat(rows_e[dst_order], seg_starts, axis=0)
        out = np.zeros((n, rows_e.shape[1]), np.float32)
        out[seg_nodes] = sums
        return out

    def bc(idx, w):
        return np.bincount(idx, weights=w, minlength=n).astype(np.float32)

    # ---------- reference math on host, device for gathers/dots ----------
    def att(feat, dots, mask):
        nrm = np.sqrt((feat ** 2).sum(1))
        safe = np.where(nrm == 0, 1.0, nrm).astype(np.float32)
        sim_p = dots / (safe[ps] * safe[pd_])
        sim_e = sim_p[pair_of_edge]            # symmetric expand to E edges
        sim_e = np.where(sim_e < 0.1, 0.0, sim_e) * mask
        rowsum = bc(src, np.abs(sim_e))
        a = sim_e / np.where(rowsum == 0, 1.0, rowsum)[src]
        z = a * Wd[0, 0] + a[rev] * Wd[1, 0] + bd[0]
        keep = 1.0 / (1.0 + np.exp(-z)) > 0.5
        a = np.where(keep, a, 0.0).astype(np.float32)
        deg = bc(src, (a != 0).astype(np.float32))
        lam = 1.0 / (deg + 1.0)
        w_e = np.where(a > 0, np.exp(a), 0.0).astype(np.float32)
        w_s = np.exp(lam).astype(np.float32)
        return w_e, w_s

    def conv(rows_e, hh, w_e, w_s, b):
        # deg over [edges ; att self loops ; conv self loops]
        degc = bc(dst, w_e) + w_s + 1.0
        dis = np.where(degc > 0, degc ** -0.5, 0.0).astype(np.float32)
        normc = dis[src] * w_e * dis[dst]
        agg = segsum(normc[:, None] * rows_e)
        agg += (dis * dis * (w_s + 1.0))[:, None] * hh
        return agg + b[None, :]

    def pad_glob(t, width):
        g = np.zeros((NC * SR + 2, width), np.float32)
        g[:n, :t.shape[1]] = t
        return g

    def pad_dtab(t, width):
        g = np.zeros((DTR, width), np.float32)
        g[:n, :t.shape[1]] = t
        return g

    # launch A: L1 dots on raw features (bf16 tables) + L3 rows of h0
    h0 = (x @ W1).astype(np.float32)
    maps = simp.in_maps(pad_glob(x, NFEAT).astype(BF16),
                        pad_dtab(x, NFEAT).astype(BF16))
    rmaps = rowp.in_maps(pad_glob(h0, NHID))
    for mm, rm in zip(maps, rmaps):
        mm["rtab"] = rm["tab"]
        mm["ridx"] = rm["idx"]
    res = run.run(nc_l1, maps)
    dots1 = simp.unwrap(res)
    rows1 = rowp.unwrap(res)
    if os.environ.get("K_DUMP"):
        np.save("/tmp/dots1.npy", dots1)
        np.save("/tmp/exp_dots1.npy", (x[ps] * x[pd_]).sum(1).astype(np.float32))
        np.save("/tmp/rows1.npy", rows1)
        np.save("/tmp/exp_rows1.npy", h0[src])
    we1, ws1 = att(x, dots1, np.ones(E, np.float32))
    h = np.maximum(conv(rows1, h0, we1, ws1, b1), 0.0).astype(np.float32)

    # launch B: L4 dots on hidden features (f32 tables) + L6 rows of h2
    h2 = (h @ W2).astype(np.float32)
    maps = simp.in_maps(pad_glob(h, NHID), pad_dtab(h, NHID))
    rmaps = rowp.in_maps(pad_glob(h2, NHID))
    for mm, rm in zip(maps, rmaps):
        mm["rtab"] = rm["tab"]
        mm["ridx"] = rm["idx"]
    res = run.run(nc_l2, maps)
    dots2 = simp.unwrap(res)
    rows2 = rowp.unwrap(res)[:, :NCLASS]
    if os.environ.get("K_DUMP"):
        np.save("/tmp/dots2.npy", dots2)
        np.save("/tmp/exp_dots2.npy", (h[ps] * h[pd_]).sum(1).astype(np.float32))
    we2, ws2 = att(h, dots2, (we1 > 0).astype(np.float32))
    out = conv(rows2, h2, we2, ws2, b2)
    mx = out.max(1, keepdims=True)
    lse = np.log(np.exp(out - mx).sum(1, keepdims=True)) + mx
    return (out - lse).astype(np.float32)
